# revision 60
# baseline (speedup 1.0000x reference)
"""BiMiniGRU Trainium2 kernel (v2: fp8 DoubleRow matmuls + fused gate DVE op).

Problem: bidirectional minimal GRU, B=8, L=8192, C=D=256.
  fwd: h[t] = z[t]*htil[t] + (1-z[t])*h[t-1],  out_f = h * sig(x@Ws+bs)
  bwd: same scanned in reverse time.
  out = out_f + out_b

Sharding: data-parallel over batch, one batch element per NeuronCore (8 cores).

Numerics: matmuls run on the PE in fp8(e4m3) DoubleRow mode (K=256 per pass,
0.5 cycles/out-col = 4x bf16 rate). To escape e4m3's subnormal zone for the
uniform(+-1/16) weights, the stationary weights are stored scaled:
  pass1: q8(x)        @ q8(512*W)
  pass2: q8(32*(x-x8))@ q8(16*W)       (x residual)
  pass3: q8(x)        @ q8(512*W - W8) (W residual)
accumulated in one fp32 PSUM at scale 512; the 1/512 is folded into the ACT
sigmoid `scale` and the custom DVE op immediate. Measured end-to-end rel err
~7e-3, slightly better than the bf16 baseline (8.4e-3).

Per-core dataflow (meet-in-the-middle over 8 chunks of 1024 timesteps; the
per-unit work is software-pipelined: stage_a = uz matmuls + a-sigmoid runs
one unit ahead of stage_b = rest, fitting the 3x[128,1024]fp32 PSUM budget):
  - x is pre-transposed/pre-quantized host-side to fp8 [2, 256, 8192]
    (hi+lo), loaded as [128, 2(var), 2(khalf), 1024] tiles by plain DMA.
  - PE: 9 DoubleRow matmuls per (dir, dt, chunk) unit (N=512 out cols per
    instruction, the ISA moving-size limit); PSUM [128,1024] fp32.
  - ACT: a = sig(-uz/512 - bz), s = sig(us/512 + bs) from PSUM, bf16 out.
  - DVE: custom op BFORM_ANT b = (uh/512 + bh)*(1 - a)  (one 1x pass from
    PSUM; kills the separate z=1-a and bias-add ops), then
    h = tensor_tensor_scan(a, b) chained across chunks (bwd scans
    right-to-left via step=-1 APs).
  - Pool: half = h*s as tensor_tensor (the Pool ISA has no stt/scan forms
    and cannot touch PSUM); the final step's last halves go to the DVE.
  - finalize (from step 4, two chunks per step): DVE adds half_f + half_b
    (bf16 2x), PE transposes the sum to [t, d] (PSUM bf16), ACT copy
    upcasts PSUM->SBUF fp32, plain store DMA to DRAM.
"""

import os
import sys

import numpy as np

for _p in ("/opt/trn_rl_repo", "/opt/pypackages"):
    if _p not in sys.path and os.path.isdir(_p):
        sys.path.append(_p)

import concourse.bacc as bacc
import concourse.bass as bass
import concourse.tile as tile
from concourse import mybir
from concourse.bass_utils import run_bass_kernel_spmd

F32 = mybir.dt.float32
BF16 = mybir.dt.bfloat16
F8 = mybir.dt.float8e4

B, L, C, D = 8, 8192, 256, 256
CHUNK = 1024
NSUB = CHUNK // 128       # t-subtiles per chunk
NDT = D // 128            # 2 d-tiles
AluOp = mybir.AluOpType
ActFn = mybir.ActivationFunctionType
PerfMode = mybir.MatmulPerfMode
USCALE = 512.0

# engine assignment knobs (tuned against TimelineSim)
# NOTE: the Pool engine's ISA only accepts tensor_scalar / tensor_tensor
# (walrus rejects TensorScalarPtr stt/scan forms on Pool), so Pool work
# must be expressed as tensor_tensor.
MM_N = 512                # matmul out columns per instruction (ISA limit:
                          # DoubleRow moving free size 2*MM_N <= 1024)
SCAN_ENG = "vector"       # per-unit scan engine
HALF_ENG = "gpsimd"       # half = h*s
FADD_ENG = "vector"       # osb = half_f + half_b
COPY_ENG = "scalar"       # otp(PSUM) -> ots(SBUF fp32)


# ---- custom DVE op: b = (uh*imm2 + bh) * (1 - a) --------------------------


def _bform_reference(in0, in1, s0, s1, imm2):
    u = np.asarray(in0, np.float32)
    a = np.asarray(in1, np.float32)
    bh = s0 if isinstance(s0, float) else np.asarray(s0, np.float32)
    return (u * imm2 + bh) * (1.0 - a)


_BFORM = None


def _get_bform():
    global _BFORM
    if _BFORM is not None:
        return _BFORM
    import concourse.dve_ops as dvo
    from concourse.dve_spec import C0, C2, One, Spec, Src0, Src1, lower
    from concourse.dve_uop import DveOpSpec

    name = "BFORM_ANT"
    if name in dvo._SUB_OPCODE_FOR_NAME:
        _BFORM = next(op for op in dvo.OPS if op.name == name)
        return _BFORM
    spec = Spec(
        body=(Src0 * C2 + C0) * (One - Src1),
        reference=_bform_reference,
    )
    row = dvo._CUSTOM_DVE_ROW_BASE + len(dvo.OPS)
    assert row < 0x20
    shas = {}
    for ver in ("v3", "v4"):
        uops = lower(spec, ver=ver)
        shas[ver] = DveOpSpec(name=name, opcode=row, uops=uops, rd1_en=True).sha(ver)
    op = dvo.DveOp(name, spec, subdim=False, uops_sha=shas)
    dvo.OPS.append(op)
    dvo.CUSTOM_DVE_SPECS[name] = spec
    dvo._SUB_OPCODE_FOR_NAME[name] = row
    _BFORM = op
    return op


def build_program(seq_len=L, num_devices=8):
    nc = bacc.Bacc(
        "TRN2", target_bir_lowering=False, debug=False, num_devices=num_devices
    )

    # x[variant, c, t] (variant 0 = q8(x), 1 = q8(32*(x - x8)))
    xq_d = nc.dram_tensor("xq", [2, C, seq_len], F8, kind="ExternalInput")
    # w[dir, proj, variant, kpart, khalf, m]
    w_d = nc.dram_tensor("w", [2, 3, 3, 128, 2, D], F8, kind="ExternalInput")
    # cst[p, :12] = bias cols (dt*6+dir*3+idx), [p, 12:16] = h0 cols (dt*2+dir)
    cst_d = nc.dram_tensor("cst", [128, 16], F32, kind="ExternalInput")
    ident_d = nc.dram_tensor("ident", [128, 128], BF16, kind="ExternalInput")
    out_d = nc.dram_tensor("out", [seq_len, D], F32, kind="ExternalOutput")

    with tile.TileContext(nc) as tc:
        _body(
            nc, tc, xq_d.ap(), w_d.ap(), cst_d.ap(), ident_d.ap(), out_d.ap(),
            seq_len,
        )
    nc.compile()
    return nc


def _body(nc, tc, xq_ap, w_ap, cst_ap, ident_ap, out_ap, seq_len=L):
    from contextlib import ExitStack

    bform = _get_bform()
    nch = seq_len // CHUNK
    ctx = ExitStack()
    with ctx:
        const_pool = ctx.enter_context(tc.tile_pool(name="const", bufs=1))
        xts_pool = ctx.enter_context(tc.tile_pool(name="xts", bufs=8))
        u_pool = ctx.enter_context(tc.tile_pool(name="u", bufs=3, space="PSUM"))
        gate_pool = ctx.enter_context(tc.tile_pool(name="gate", bufs=13))
        h_pool = ctx.enter_context(tc.tile_pool(name="h", bufs=10))
        half_pool = ctx.enter_context(tc.tile_pool(name="half", bufs=12))
        osb_pool = ctx.enter_context(tc.tile_pool(name="osb", bufs=4))
        otp_pool = ctx.enter_context(tc.tile_pool(name="otp", bufs=2, space="PSUM"))
        ots_pool = ctx.enter_context(tc.tile_pool(name="ots", bufs=2))

        # ---- persistent constants (3 DMAs, issued off the ACT queue so the
        # first x loads on SP aren't serialized behind them; weights first
        # since they gate the first matmul, ident last — it's only needed at
        # the first finalize, 4 steps in) ----
        # weights in four DMAs ordered by first use (dir0-uz gates the very
        # first matmul): tile [128, 18(di pj v), 2, 256] fp8
        wt = const_pool.tile([128, 18, 2, D], F8)
        w_v = w_ap.rearrange("di pj v p i m -> p (di pj v) i m")
        for sl in (slice(3, 6), slice(0, 3), slice(6, 9), slice(9, 18)):
            nc.scalar.dma_start(wt[:, sl], w_v[:, sl])
        w_sb = {
            (di, pj, v): wt[:, (di * 3 + pj) * 3 + v]
            for di in range(2) for pj in range(3) for v in range(3)
        }

        # bias+h0 in one DMA: [128, 16]
        cst_sb = const_pool.tile([128, 16], F32)
        nc.scalar.dma_start(cst_sb[:], cst_ap[:, :])

        ident = const_pool.tile([128, 128], BF16)
        nc.scalar.dma_start(ident[:], ident_ap[:, :])

        def bias_col(dt_i, di, idx):
            return cst_sb[:, dt_i * 6 + di * 3 + idx : dt_i * 6 + di * 3 + idx + 1]

        # warm the ACT sigmoid table set during the prologue DMAs, with the
        # same operand shape as the real sigmoids (bias AP + scale)
        warm = const_pool.tile([128, 1], BF16)
        nc.scalar.activation(
            warm[:], cst_sb[:, 12:13], ActFn.Sigmoid,
            bias=bias_col(0, 0, 1), scale=-1.0 / USCALE,
        )

        def h0_col(dt_i, di):
            return cst_sb[:, 12 + dt_i * 2 + di : 12 + dt_i * 2 + di + 1]

        def eng(name):
            return getattr(nc, name)

        # x DRAM [2, 256, seq] viewed as [128, variant, khalf, seq]
        xq_v = xq_ap.rearrange("v (i p) t -> p v i t", p=128)

        def load_chunk(c, split=False):
            """Load x hi+lo for chunk c as one [128, 2, 2, CHUNK] fp8 tile.

            split=True issues one DMA per variant (parallel transfers, and
            the hi part — needed by the first two matmul passes — lands
            first); used for the prologue loads on the critical path.
            """
            t = xts_pool.tile([128, 2, 2, CHUNK], F8, tag="xt")
            sl = slice(c * CHUNK, (c + 1) * CHUNK)
            if split:
                nc.sync.dma_start(t[:, 0], xq_v[:, 0, :, sl])
                nc.sync.dma_start(t[:, 1], xq_v[:, 1, :, sl])
            else:
                nc.sync.dma_start(t[:], xq_v[:, :, :, sl])
            return t

        half_f = {}
        half_b = {}
        h_prev = {}  # (dir, dt) -> h tile of previous chunk in stream order

        def mm(di, pj, xt, dt_i):
            msl = slice(dt_i * 128, (dt_i + 1) * 128)
            up = u_pool.tile([128, CHUNK], F32, tag="u")
            # variant-outer order: consecutive matmuls share a stationary
            for v in (0, 1, 2):
                xv = xt[:, 1 if v == 1 else 0]
                for nh in range(CHUNK // MM_N):
                    osl = slice(nh * MM_N, (nh + 1) * MM_N)
                    nc.tensor.matmul(
                        up[:, osl],
                        w_sb[(di, pj, v)][:, :, msl],
                        xv[:, :, osl],
                        start=(v == 0),
                        stop=(v == 2),
                        perf_mode=PerfMode.DoubleRow,
                    )
            return up

        def stage_a(di, c, xt, dt_i):
            """uz matmuls + a-sigmoid (issued one unit ahead of stage_b)."""
            uz = mm(di, 1, xt, dt_i)
            a_t = gate_pool.tile([128, CHUNK], BF16, tag="a")
            nc.scalar.activation(
                a_t[:], uz[:], ActFn.Sigmoid,
                bias=bias_col(dt_i, di, 1), scale=-1.0 / USCALE,
            )
            return a_t

        def stage_b(di, c, reverse_time, xt, half, dt_i, a_t, half_eng=HALF_ENG):
            # b = (uh/512 + bh) * (1 - a)   (fused custom DVE op) — emitted
            # before the s-path so the DVE bform/scan chain starts early
            uh = mm(di, 0, xt, dt_i)
            b_t = gate_pool.tile([128, CHUNK], BF16, tag="b")
            nc.vector._custom_dve(
                bform, out=b_t[:], in0=uh[:], in1=a_t[:],
                s0=bias_col(dt_i, di, 0), imm2=1.0 / USCALE,
            )
            # s = sigmoid(us/512 + bs)
            us = mm(di, 2, xt, dt_i)
            s_t = gate_pool.tile([128, CHUNK], BF16, tag="s")
            nc.scalar.activation(
                s_t[:], us[:], ActFn.Sigmoid,
                bias=bias_col(dt_i, di, 2), scale=1.0 / USCALE,
            )
            # h = scan(a, b): h[t] = a[t]*h[t-1] + b[t]
            h_t = h_pool.tile([128, CHUNK], BF16, tag="h")
            prev = h_prev.get((di, dt_i))
            if prev is None:
                init = h0_col(dt_i, di)
            elif reverse_time:
                init = prev[:, 0:1]
            else:
                init = prev[:, CHUNK - 1 : CHUNK]
            if reverse_time:
                eng(SCAN_ENG).tensor_tensor_scan(
                    h_t[:, ::-1], a_t[:, ::-1], b_t[:, ::-1], init,
                    op0=AluOp.mult, op1=AluOp.add,
                )
            else:
                eng(SCAN_ENG).tensor_tensor_scan(
                    h_t[:], a_t[:], b_t[:], init,
                    op0=AluOp.mult, op1=AluOp.add,
                )
            h_prev[(di, dt_i)] = h_t
            # half = h * s
            eng(half_eng).tensor_tensor(
                half[:, dt_i * CHUNK : (dt_i + 1) * CHUNK],
                h_t[:], s_t[:], op=AluOp.mult,
            )

        def finalize_chunk(c, fadd_eng=FADD_ENG, last=False):
            """out[c] = half_f[c] + half_b[c]; transpose to [t,d]; store."""
            hf = half_f.pop(c)
            hb = half_b.pop(c)
            osb = []
            for dt_i in range(NDT):
                o = osb_pool.tile([128, CHUNK], BF16, tag="osb")
                eng(fadd_eng).tensor_tensor(
                    o[:],
                    hf[:, dt_i * CHUNK : (dt_i + 1) * CHUNK],
                    hb[:, dt_i * CHUNK : (dt_i + 1) * CHUNK],
                    op=AluOp.add,
                )
                osb.append(o)
            # transpose+copy+store in two 512-timestep halves (otp = 1 bank);
            # the PSUM->SBUF fp32 upcast copies alternate ACT / Pool
            for hh in range(NSUB // 4):
                otp = otp_pool.tile([128, 4 * D], BF16, tag="otp")
                for s in range(4):
                    s_abs = hh * 4 + s
                    for dt_i in range(NDT):
                        nc.tensor.transpose(
                            otp[:, s * D + dt_i * 128 : s * D + (dt_i + 1) * 128],
                            osb[dt_i][:, s_abs * 128 : (s_abs + 1) * 128],
                            ident[:],
                        )
                # copy upcasts bf16->fp32 in the same pass (per-element
                # cost). GPSIMD can't read PSUM, so ACT normally; in the
                # final drain the DVE is idle, so split the last copies.
                ots = ots_pool.tile([128, 4 * D], F32, tag="ots")
                if last and hh == 1:
                    nc.vector.tensor_scalar(
                        ots[:], otp[:], 1.0, None, AluOp.mult,
                    )
                else:
                    nc.scalar.copy(ots[:], otp[:])
                dst = out_ap[
                    c * CHUNK + hh * 512 : c * CHUNK + (hh + 1) * 512, :
                ].rearrange("(s p) d -> p s d", p=128)
                nc.sync.dma_start(dst, ots[:].rearrange("p (s d) -> p s d", d=D))

        # prologue: first step's x tiles (each chunk is loaded once per
        # consuming direction; fwd uses chunk k at step k, bwd uses chunk
        # nch-1-k, so a chunk is re-loaded when its second direction comes up)
        loaded = {}

        def load_once(c):
            if c not in loaded:
                loaded[c] = load_chunk(c)
            return loaded.pop(c)

        def preload(c):
            if 0 <= c < nch and c not in loaded:
                loaded[c] = load_chunk(c)

        loaded[0] = load_chunk(0, split=True)
        loaded[nch - 1] = load_chunk(nch - 1, split=True)
        # software pipeline across units: stage_a (uz matmuls + a-sigmoid)
        # runs one unit ahead of stage_b (rest), so `a` is ready when the
        # DVE bform needs it and the PE never waits on a fresh PSUM bank.
        pend = None  # (stage_b args..., a_t) for the previous unit

        def run_unit(di, c, reverse_time, xt, half, dt_i, half_eng=HALF_ENG):
            nonlocal pend
            a_t = stage_a(di, c, xt, dt_i)
            if pend is not None:
                stage_b(*pend)
            pend = (di, c, reverse_time, xt, half, dt_i, a_t, half_eng)

        def flush_unit():
            nonlocal pend
            if pend is not None:
                stage_b(*pend)
                pend = None

        fin_pending = None
        for k in range(nch):
            if k + 1 < nch:
                preload(k + 1)
                preload(nch - 2 - k)
            cf = k
            cb = nch - 1 - k
            xt_f = load_once(cf)
            xt_b = load_once(cb) if cb != cf else xt_f
            hf_t = half_pool.tile([128, 2 * CHUNK], BF16, tag="half")
            hb_t = half_pool.tile([128, 2 * CHUNK], BF16, tag="half")
            half_f[cf] = hf_t
            half_b[cb] = hb_t
            # fwd units first, then bwd: chunk cf's finalize only needs this
            # step's fwd halves (its bwd half is old), and chunk cb's only
            # needs this step's bwd halves, so each can be emitted as soon as
            # the relevant direction's stage_b ops are flushed.
            # the last bwd half feeds a same-step F-add at the step end;
            # computing just that one on the DVE (2x, in-queue right before
            # the F) avoids waiting on the slower Pool for the final tile
            last_half_eng = "vector" if k >= nch // 2 else HALF_ENG
            run_unit(0, cf, False, xt_f, hf_t, 0)
            run_unit(0, cf, False, xt_f, hf_t, 1)
            run_unit(1, cb, True, xt_b, hb_t, 0)
            run_unit(1, cb, True, xt_b, hb_t, 1, last_half_eng)
            flush_unit()
            if k >= nch // 2:
                finalize_chunk(k, last=(k == nch - 1))
                finalize_chunk(nch - 1 - k, last=(k == nch - 1))


_CACHED = {}


def _get_program():
    if "nc" not in _CACHED:
        _CACHED["nc"] = build_program()
    return _CACHED["nc"]


def _pack_inputs(inputs):
    import ml_dtypes

    f32 = np.float32
    bf16 = ml_dtypes.bfloat16
    f8 = ml_dtypes.float8_e4m3

    def q8(v):
        return v.astype(f8).astype(f32)

    # weights packed for DoubleRow: w[di, pj, v, p, i, m] = Wv[i*128+p, m]
    w = np.empty((2, 3, 3, 128, 2, D), dtype=f8)
    names = [
        [("Wh1", "bh1"), ("Wz1", "bz1"), ("Ws1", "bs1")],
        [("Wh_1", "bh_1"), ("Wz_1", "bz_1"), ("Ws_1", "bs_1")],
    ]
    for di in range(2):
        for pj in range(3):
            Wf = np.asarray(inputs[names[di][pj][0]], f32)  # [C, D]
            W512 = q8(512.0 * Wf)
            variants = (W512, q8(16.0 * Wf), 512.0 * Wf - W512)
            for v, Wv in enumerate(variants):
                w[di, pj, v] = (
                    np.asarray(Wv, f32).reshape(2, 128, D).transpose(1, 0, 2).astype(f8)
                )
    bias = np.stack(
        [
            inputs["bh1"], -np.asarray(inputs["bz1"]), inputs["bs1"],
            inputs["bh_1"], -np.asarray(inputs["bz_1"]), inputs["bs_1"],
        ],
        axis=1,
    ).astype(f32)  # [256, 6]
    h0 = np.stack(
        [np.asarray(inputs["h01"]).reshape(D), np.asarray(inputs["h0_1"]).reshape(D)],
        axis=1,
    ).astype(f32)  # [256, 2]
    # cst[p, dt*6+dir*3+idx] = bias, cst[p, 12+dt*2+dir] = h0
    cst = np.empty((128, 16), f32)
    for dt_i in range(NDT):
        cst[:, dt_i * 6 : (dt_i + 1) * 6] = bias[dt_i * 128 : (dt_i + 1) * 128]
        cst[:, 12 + dt_i * 2 : 12 + (dt_i + 1) * 2] = h0[
            dt_i * 128 : (dt_i + 1) * 128
        ]
    ident = np.eye(128).astype(bf16)
    return w, cst, ident


def kernel(**inputs):
    import ml_dtypes

    f8 = ml_dtypes.float8_e4m3
    nc = _get_program()
    w, cst, ident = _pack_inputs(inputs)
    xs = np.asarray(inputs["xs"], dtype=np.float32)
    in_maps = []
    for b in range(B):
        xt = np.ascontiguousarray(xs[b].T)          # [C, L] fp32
        xq = np.empty((2, C, L), f8)
        xq[0] = xt.astype(f8)
        xq[1] = (32.0 * (xt - xq[0].astype(np.float32))).astype(f8)
        in_maps.append(
            {
                "xq": xq,
                "w": w,
                "cst": np.ascontiguousarray(cst),
                "ident": ident,
            }
        )
    trace = bool(int(os.environ.get("KERNEL_TRACE", "0")))
    res = run_bass_kernel_spmd(nc, in_maps, core_ids=list(range(B)), trace=trace)
    if trace:
        _CACHED["last_results"] = res
    out = np.stack([res.results[b]["out"] for b in range(B)]).astype(np.float32)
    return out
